# revision 1
# baseline (speedup 1.0000x reference)
"""Trainium2 Bass kernel for nn_ComplexMixture: weighted complex Gram matrices.

Reference (per batch b, inputs real/imag [B,T,D] f32, weight [B,T,1] f32):
    out_r[b] = sum_t w[b,t] * (r_t r_t^T + i_t i_t^T)   (symmetric)
    out_i[b] = sum_t w[b,t] * (i_t r_t^T - r_t i_t^T)   (antisymmetric)
with B=64, T=256, D=512; outputs (out_r, out_i), each [B, D, D] f32.

Strategy: pure data-parallel over 8 NeuronCores (8 batches per core).
Per core, per batch:
  - load r, i tiles [128, 2*512] (T on partitions, 2 K-tiles of 128)
  - since w >= 0: a = fp16(sqrt(w)*r), c = fp16(sqrt(w)*i); fp16 matmuls
    stream 2 cols/cycle on the PE and accumulate exactly in fp32 PSUM
    (~3e-4 global rel err; fp32r would be 1.5e-4 but 3.4x slower per MM,
    plain fp32 is 7.6x slower per MM)
  - out_r = a^T a + c^T c ; out_i = c^T a + (-a)^T c  (na = -a avoids
    the missing matmul-subtract)
  - 4 accumulating matmuls per [128,512] PSUM bank, 8 banks per batch
  - evict PSUM->SBUF on DVE only: ACT PSUM copies measured slow and
    gated the output DMAs (all-DVE evictions won 149us vs 215us in-window)
  - input loads via HWDGE (sync); output stores via SWDGE (gpsimd):
    HWDGE writes measured ~3x slower for this pattern on this platform
"""
import numpy as np
from contextlib import ExitStack

import concourse.bacc as bacc
import concourse.tile as tile
from concourse import mybir
from concourse.bass_utils import run_bass_kernel_spmd

F32 = mybir.dt.float32
F32R = mybir.dt.float32r
BF16 = mybir.dt.bfloat16
FP16 = mybir.dt.float16

N_CORES = 8
B_FULL = 64
BPC = B_FULL // N_CORES  # batches per core
T, D = 256, 512
KT = T // 128            # K tiles per batch
MT = D // 128            # M tiles per output row-block

MM_DTYPE = FP16
TRI = False              # trapezoid loses: narrow DMAs cost more than bytes saved


def build_nc(reps: int = 1, mm_dtype=None, tri=None, out_engine="gpsimd",
             in_engine="sync", na_engine="gpsimd", inp_bufs=4, outp_bufs=4,
             out_mode="batch", evict_engine="dve"):
    """Build + compile the per-core program. `reps` > 1 wraps the whole body
    in a hardware loop (for timing only; output is idempotent).

    Output DMAs default to SWDGE (gpsimd): HWDGE writes measured ~3x slower
    for this pattern. gpsimd Q7 also generates SWDGE descriptors, so the na
    negation runs on DVE by default to keep gpsimd free for DMA."""
    if mm_dtype is None:
        mm_dtype = MM_DTYPE
    if tri is None:
        tri = TRI
    nc = bacc.Bacc("TRN2", target_bir_lowering=False, debug=False)
    real = nc.dram_tensor("real", [BPC, T, D], F32, kind="ExternalInput").ap()
    imag = nc.dram_tensor("imag", [BPC, T, D], F32, kind="ExternalInput").ap()
    weight = nc.dram_tensor("weight", [BPC, T, 1], F32, kind="ExternalInput").ap()
    out_r = nc.dram_tensor("out_r", [BPC, D, D], F32, kind="ExternalOutput").ap()
    out_i = nc.dram_tensor("out_i", [BPC, D, D], F32, kind="ExternalOutput").ap()

    # trapezoid geometry: row-block mi covers output cols [mi*128, 512)
    widths = [D - 128 * mi for mi in range(MT)] if tri else [D] * MT
    offs = [sum(widths[:j]) for j in range(MT)]
    stage_w = sum(widths)

    with tile.TileContext(nc) as tc, ExitStack() as ctx:
        wp = ctx.enter_context(tc.tile_pool(name="wp", bufs=1))
        inp = ctx.enter_context(tc.tile_pool(name="inp", bufs=inp_bufs))
        wgt = ctx.enter_context(tc.tile_pool(name="wgt", bufs=3))
        outp = ctx.enter_context(tc.tile_pool(name="outp", bufs=outp_bufs))
        psp = ctx.enter_context(tc.tile_pool(name="psp", bufs=8, space="PSUM"))

        if out_engine == "split":
            out_dma_r, out_dma_i = nc.gpsimd.dma_start, nc.sync.dma_start
        elif out_engine == "sync":
            out_dma_r = out_dma_i = nc.sync.dma_start
        else:
            out_dma_r = out_dma_i = nc.gpsimd.dma_start
        out_dma = out_dma_r
        in_dma = nc.sync.dma_start if in_engine == "sync" else nc.gpsimd.dma_start
        na_mul = nc.vector.tensor_scalar_mul if na_engine == "vector" else nc.gpsimd.tensor_scalar_mul

        def body(_iv=None):
            # all weights for the core's batches: W[p, b*KT+kt] = w[b, kt*128+p]
            W = wp.tile([128, BPC * KT], F32, tag="W")
            nc.sync.dma_start(
                W[:], weight.rearrange("b (kt p) o -> p (b kt o)", kt=KT, p=128)
            )
            SW = wp.tile([128, BPC * KT], F32, tag="SW")
            nc.scalar.activation(SW[:], W[:], mybir.ActivationFunctionType.Sqrt)
            NSW = wp.tile([128, BPC * KT], F32, tag="NSW")
            nc.vector.tensor_scalar_mul(NSW[:], SW[:], -1.0)

            for b in range(BPC):
                rt = inp.tile([128, KT * D], F32, tag="rt")
                it = inp.tile([128, KT * D], F32, tag="it")
                for kt in range(KT):
                    sl = slice(kt * D, (kt + 1) * D)
                    in_dma(rt[:, sl], real[b, kt * 128:(kt + 1) * 128, :])
                    in_dma(it[:, sl], imag[b, kt * 128:(kt + 1) * 128, :])

                a = wgt.tile([128, KT * D], mm_dtype, tag="a")   # sqrt(w)*r
                c = wgt.tile([128, KT * D], mm_dtype, tag="c")   # sqrt(w)*i
                na = wgt.tile([128, KT * D], mm_dtype, tag="na")  # -sqrt(w)*r
                for kt in range(KT):
                    sl = slice(kt * D, (kt + 1) * D)
                    ws = SW[:, b * KT + kt:b * KT + kt + 1]
                    nws = NSW[:, b * KT + kt:b * KT + kt + 1]
                    nc.vector.tensor_scalar_mul(a[:, sl], rt[:, sl], ws)
                    nc.vector.tensor_scalar_mul(c[:, sl], it[:, sl], ws)
                    na_mul(na[:, sl], rt[:, sl], nws)

                or_sb = outp.tile([128, stage_w], F32, tag="or")
                oi_sb = outp.tile([128, stage_w], F32, tag="oi")
                for mi in range(MT):
                    w_mi = widths[mi]
                    pr = psp.tile([128, w_mi], F32, tag="ps", padded_shape=[128, D], name="pr")
                    pi = psp.tile([128, w_mi], F32, tag="ps", padded_shape=[128, D], name="pi")
                    col0 = mi * 128 if tri else 0
                    for kt in range(KT):
                        m = slice(kt * D + mi * 128, kt * D + mi * 128 + 128)
                        n = slice(kt * D + col0, kt * D + col0 + w_mi)
                        st = kt == 0
                        nc.tensor.matmul(pr[:], a[:, m], a[:, n], start=st, stop=False)
                        nc.tensor.matmul(pi[:], c[:, m], a[:, n], start=st, stop=False)
                    for kt in range(KT):
                        m = slice(kt * D + mi * 128, kt * D + mi * 128 + 128)
                        n = slice(kt * D + col0, kt * D + col0 + w_mi)
                        sp = kt == KT - 1
                        nc.tensor.matmul(pr[:], c[:, m], c[:, n], start=False, stop=sp)
                        nc.tensor.matmul(pi[:], na[:, m], c[:, n], start=False, stop=sp)
                    osl = slice(offs[mi], offs[mi] + w_mi)
                    if evict_engine == "dve":
                        nc.vector.tensor_copy(or_sb[:, osl], pr[:])
                        nc.vector.tensor_copy(oi_sb[:, osl], pi[:])
                    elif mi % 2 == 0:
                        nc.scalar.copy(or_sb[:, osl], pr[:])
                        nc.vector.tensor_copy(oi_sb[:, osl], pi[:])
                    else:
                        nc.vector.tensor_copy(or_sb[:, osl], pr[:])
                        nc.scalar.copy(oi_sb[:, osl], pi[:])
                if tri:
                    for mi in range(MT):
                        osl = slice(offs[mi], offs[mi] + widths[mi])
                        rows = slice(mi * 128, (mi + 1) * 128)
                        cols = slice(mi * 128, D)
                        out_dma_r(out_r[b, rows, cols], or_sb[:, osl])
                        out_dma_i(out_i[b, rows, cols], oi_sb[:, osl])
                elif out_mode == "mi":
                    # fine-grained: one contiguous 256KB DMA per row-block,
                    # issued as soon as each eviction lands (smoother write
                    # pacing, smaller bursts)
                    for mi in range(MT):
                        rows = slice(mi * 128, (mi + 1) * 128)
                        osl = slice(mi * D, (mi + 1) * D)
                        out_dma_r(out_r[b, rows, :], or_sb[:, osl])
                        out_dma_i(out_i[b, rows, :], oi_sb[:, osl])
                else:
                    out_dma_r(
                        out_r[b].rearrange("(mi p) c -> p mi c", mi=MT, p=128),
                        or_sb[:].rearrange("p (mi c) -> p mi c", mi=MT, c=D),
                    )
                    out_dma_i(
                        out_i[b].rearrange("(mi p) c -> p mi c", mi=MT, p=128),
                        oi_sb[:].rearrange("p (mi c) -> p mi c", mi=MT, c=D),
                    )

        if reps == 1:
            body()
        else:
            with tc.For_i(0, reps, 1) as iv:
                body(iv)

    nc.compile()
    return nc


_NC_CACHE = {}


def _get_nc(reps: int = 1):
    key = (reps, MM_DTYPE, TRI)
    if key not in _NC_CACHE:
        _NC_CACHE[key] = build_nc(reps=reps)
    return _NC_CACHE[key]


def _mirror(out_r, out_i):
    """Fill lower-triangle blocks from the device-computed upper trapezoid:
    out_r symmetric, out_i antisymmetric."""
    for mi in range(1, MT):
        for nj in range(mi):
            rs, cs = slice(mi * 128, (mi + 1) * 128), slice(nj * 128, (nj + 1) * 128)
            out_r[:, rs, cs] = out_r[:, cs, rs].transpose(0, 2, 1)
            out_i[:, rs, cs] = -out_i[:, cs, rs].transpose(0, 2, 1)


def kernel(real, imag, weight):
    real = np.ascontiguousarray(np.asarray(real, dtype=np.float32))
    imag = np.ascontiguousarray(np.asarray(imag, dtype=np.float32))
    weight = np.ascontiguousarray(np.asarray(weight, dtype=np.float32))
    assert real.shape == (B_FULL, T, D) and weight.shape == (B_FULL, T, 1)

    nc = _get_nc()
    in_maps = [
        {
            "real": real[i * BPC:(i + 1) * BPC],
            "imag": imag[i * BPC:(i + 1) * BPC],
            "weight": weight[i * BPC:(i + 1) * BPC],
        }
        for i in range(N_CORES)
    ]
    res = run_bass_kernel_spmd(nc, in_maps, list(range(N_CORES)))
    out_r = np.concatenate([res.results[i]["out_r"] for i in range(N_CORES)], axis=0)
    out_i = np.concatenate([res.results[i]["out_i"] for i in range(N_CORES)], axis=0)
    if TRI:
        _mirror(out_r, out_i)
    return (out_r, out_i)

